# revision 1
# baseline (speedup 1.0000x reference)
"""Trainium2 Bass kernel for nn_CrossNetwork (DCN-v1 cross network).

Math: reference computes x_{i+1} = input * (x_i . w_i) + x_i + b_i, L=6 layers.
Writing x_i = input * c_i + B_i with B_i = sum_{j<i} b_j (a constant row
vector) and c_i a per-row scalar, the recursion collapses to
    u_i    = input . w_i                     (per row, one tall-skinny matmul)
    beta_i = B_i . w_i                       (host-computed constants)
    c_{i+1} = c_i * (1 + u_i) + beta_i ; c_0 = 1
    out    = input * c_L + B_L
For the b == 0 case this is out = input * prod_i(1 + u_i).

Device work per core (2048 rows): load x once, PE-transpose 128x128 blocks,
fp32 matmul against W^T accumulating U[rows, 6], DVE product-reduce to c,
DVE per-partition-scalar multiply, store. ~2 passes over HBM => memory-bound.
"""

import numpy as np

import concourse.bass as bass
import concourse.mybir as mybir
import concourse.tile as tile
from concourse.bass_utils import run_bass_kernel_spmd
from concourse.masks import make_identity
from concourse.vector_clock import ScopedClock

F32 = mybir.dt.float32

B, D, L = 16384, 1024, 6
NCORES = 8
R = B // NCORES  # rows per core
P = 128
NCH = R // P  # chunks of 128 rows per core
KB = D // P  # 128-wide k blocks
GRP = 4  # chunks per psum accumulation group
DMA_SPLIT = False  # SP HWDGE alone measured faster (68 vs 76 us)
MM_F32R = False  # use float32r (full-rate) matmuls instead of fp32
NG = NCH // GRP


def _patch_tile_drain():
    """This walrus build rejects >1 sem wait on a CTRL (Drain) instruction.

    Tile's kernel-tail drain waits on every sem domain at once; split it into
    chained single-wait drains.
    """
    if getattr(tile.TileContext, "_drain_patched", False):
        return

    def _drain_and_barrier(self, tick_clock, wait_clock):
        gc = tick_clock.global_clock
        entries = [(proc, t) for proc, t in enumerate(gc) if t > 0]
        if not entries:
            self.nc.sync.drain()
        for proc, t in entries:
            sub = ScopedClock()
            sub.require_at_least(None, proc, t)
            drain_inst = self.nc.sync.drain()
            wait_clock.add_sem_waits(drain_inst.ins, sub)

        self.nc.all_engine_barrier()
        assert self.sems is not None
        popped = self.nc._tile_sem_poison_stack.pop()
        assert popped is self._sem_poison
        self.nc.clear_and_free_semaphores(list(self.sems.allocated().values()))

    tile.TileContext._drain_and_barrier = _drain_and_barrier
    tile.TileContext._drain_patched = True


def _build(with_bias: bool, loop_n: int = 1, mode: str = "full"):
    nc = bass.Bass("TRN2")
    x_d = nc.dram_tensor("x", [R, D], F32, kind="ExternalInput")
    wt_d = nc.dram_tensor("wt", [D, L], F32, kind="ExternalInput")
    if with_bias:
        bl_d = nc.dram_tensor("bl", [1, D], F32, kind="ExternalInput")
        beta_d = nc.dram_tensor("beta", [1, L], F32, kind="ExternalInput")
    y_d = nc.dram_tensor("y", [R, D], F32, kind="ExternalOutput")

    xv = x_d.rearrange("(p n) d -> p n d", p=P)  # [128, NCH, D]
    yv = y_d.rearrange("(p n) d -> p n d", p=P)
    wtv = wt_d.rearrange("(k p) s -> p k s", p=P)  # [128, KB, L]

    with tile.TileContext(nc) as tc:
        with (
            tc.tile_pool(name="consts", bufs=1) as consts,
            tc.tile_pool(name="xch", bufs=NCH) as xpool,
            tc.tile_pool(name="xt", bufs=6) as xtpool,
            tc.tile_pool(name="small", bufs=2 * GRP) as small,
            tc.tile_pool(name="pxt", bufs=4, space="PSUM") as pxt,
            tc.tile_pool(name="pu", bufs=2, space="PSUM") as pu,
        ):
            ident = consts.tile([P, P], F32)
            make_identity(nc, ident)
            ident6 = consts.tile([L, L], F32)
            make_identity(nc, ident6)
            wt_sb = consts.tile([P, KB, L], F32)
            nc.sync.dma_start(out=wt_sb, in_=wtv)
            if with_bias:
                bl_sb = consts.tile([P, D], F32)
                nc.sync.dma_start(
                    out=bl_sb,
                    in_=bass.AP(tensor=bl_d, offset=0, ap=[[0, P], [1, D]]),
                )
                beta_sb = consts.tile([P, L], F32)
                nc.sync.dma_start(
                    out=beta_sb,
                    in_=bass.AP(tensor=beta_d, offset=0, ap=[[0, P], [1, L]]),
                )

            import contextlib
            loop_cm = (
                tc.For_i(0, loop_n, 1) if loop_n > 1 else contextlib.nullcontext()
            )
            if mode == "compute":
                x_pre = []
                for n in range(NCH):
                    xt_pre = xpool.tile([P, D], F32, tag="xch", name=f"xpre{n}")
                    nc.sync.dma_start(out=xt_pre, in_=xv[:, n, :])
                    x_pre.append(xt_pre)
            else:
                x_pre = None
            with loop_cm:
                _body(nc, tc, xpool, xtpool, small, pxt, pu, consts, ident,
                      ident6, wt_sb, locals().get("bl_sb"),
                      locals().get("beta_sb"), xv, yv, with_bias, mode, x_pre)
    return nc


def _body(nc, tc, xpool, xtpool, small, pxt, pu, consts, ident, ident6,
          wt_sb, bl_sb, beta_sb, xv, yv, with_bias, mode="full", x_pre=None):
            if mode == "compute":
                x_sb = x_pre
            else:
                x_sb = []
                for n in range(NCH):
                    xt_tile = xpool.tile([P, D], F32, tag="xch")
                    eng = nc.sync if (not DMA_SPLIT or n % 2 == 0) else nc.scalar
                    eng.dma_start(out=xt_tile, in_=xv[:, n, :])
                    x_sb.append(xt_tile)
            if mode == "dma":
                for n in range(NCH):
                    eng = nc.scalar if (DMA_SPLIT and n % 2 == 0) else nc.sync
                    eng.dma_start(out=yv[:, n, :], in_=x_sb[n])
                return

            for g in range(NG):
                # U^T[6, 512] accumulated over k blocks; stationary weights
                # are only 6 columns so LDWEIGHTS is trivial.
                ut_ps = pu.tile([L, GRP * P], F32, tag="u", name=f"ut{g}")
                for k in range(KB):
                    pxt_t = pxt.tile([P, GRP * P], F32, tag="pxt")
                    for j in range(GRP):
                        nc.tensor.transpose(
                            pxt_t[:, j * P : (j + 1) * P],
                            x_sb[g * GRP + j][:, k * P : (k + 1) * P],
                            ident,
                        )
                    xt_t = xtpool.tile([P, GRP * P], F32, tag="xt")
                    nc.scalar.copy(xt_t, pxt_t)
                    lhs = wt_sb[:, k, :]
                    rhs = xt_t[:]
                    if MM_F32R:
                        lhs = lhs.bitcast(mybir.dt.float32r)
                        rhs = rhs.bitcast(mybir.dt.float32r)
                    nc.tensor.matmul(
                        ut_ps[:],
                        lhs,
                        rhs,
                        start=(k == 0),
                        stop=(k == KB - 1),
                    )
                # 1 + U^T on ACT while copying PSUM->SBUF, then transpose
                # [6,128] blocks back to row-major [128,6] per chunk.
                u1t_t = xtpool.tile([L, GRP * P], F32, tag="u1t")
                nc.vector.tensor_scalar_add(u1t_t, ut_ps, 1.0)
                uj_ps = pu.tile([P, GRP, L], F32, tag="uj", name=f"uj{g}")
                for j in range(GRP):
                    nc.tensor.transpose(
                        uj_ps[:, j, :],
                        u1t_t[:, j * P : (j + 1) * P],
                        ident6,
                    )
                for j in range(GRP):
                    n = g * GRP + j
                    u1_t = small.tile([P, L], F32, tag="u1")
                    nc.vector.tensor_copy(u1_t, uj_ps[:, j, :])
                    if not with_bias:
                        # c = prod over the 6 (1+u_i): 3 pairwise muls
                        p3_t = small.tile([P, 3], F32, tag="p3")
                        nc.vector.tensor_mul(p3_t, u1_t[:, 0:3], u1_t[:, 3:6])
                        p1_t = small.tile([P, 1], F32, tag="p1")
                        nc.vector.tensor_mul(p1_t, p3_t[:, 0:1], p3_t[:, 1:2])
                        c_t = small.tile([P, 1], F32, tag="c")
                        nc.vector.tensor_mul(c_t, p1_t, p3_t[:, 2:3])
                        if mode == "compute":
                            scr_t = xtpool.tile([P, D], F32, tag="scr")
                            nc.vector.tensor_scalar_mul(scr_t, x_sb[n], c_t)
                        else:
                            nc.vector.tensor_scalar_mul(x_sb[n], x_sb[n], c_t)
                    else:
                        c_t = small.tile([P, 1], F32, tag="c")
                        nc.vector.memset(c_t, 1.0)
                        for i in range(L):
                            # c = c * (1 + u_i) + beta_i
                            nc.vector.scalar_tensor_tensor(
                                out=c_t,
                                in0=c_t,
                                scalar=u1_t[:, i : i + 1],
                                in1=beta_sb[:, i : i + 1],
                                op0=mybir.AluOpType.mult,
                                op1=mybir.AluOpType.add,
                            )
                        # out = x * c + B_L
                        nc.vector.scalar_tensor_tensor(
                            out=x_sb[n],
                            in0=x_sb[n],
                            scalar=c_t,
                            in1=bl_sb,
                            op0=mybir.AluOpType.mult,
                            op1=mybir.AluOpType.add,
                        )
                    if mode != "compute":
                        eng = nc.scalar if (DMA_SPLIT and n % 2 == 0) else nc.sync
                        eng.dma_start(out=yv[:, n, :], in_=x_sb[n])


def _split_multi_waits(nc):
    """This walrus build allows only one sem wait on several instruction
    structs (e.g. self-loading Matmult). Move extra waits onto preceding
    same-engine NOPs; engine FIFO order makes this equivalent."""
    n = 0
    for fn in nc.m.functions:
        for bb in fn.blocks:
            out = []
            for inst in bb.instructions:
                si = inst.sync_info
                if si is not None and si.on_wait and len(si.on_wait) > 1:
                    waits = list(si.on_wait)
                    for w in waits[:-1]:
                        n += 1
                        out.append(
                            mybir.InstNoOp(
                                name=f"nopw-{n}-{inst.name}",
                                engine=inst.engine,
                                sync_info=mybir.SyncInfo(
                                    on_wait=[w], on_update=[]
                                ),
                                bass_nofuse=True,
                            )
                        )
                    inst.sync_info = mybir.SyncInfo(
                        on_wait=[waits[-1]], on_update=list(si.on_update)
                    )
                out.append(inst)
            bb.instructions = out


_CACHE = {}


def _get_nc(with_bias: bool, loop_n: int = 1, mode: str = "full"):
    key = (with_bias, loop_n, mode, DMA_SPLIT, MM_F32R)
    if key not in _CACHE:
        _patch_tile_drain()
        nc = _build(with_bias, loop_n, mode)
        _split_multi_waits(nc)
        _CACHE[key] = nc
    return _CACHE[key]


def kernel(input, W, b, **run_kwargs):
    input = np.ascontiguousarray(np.asarray(input, dtype=np.float32))
    W = np.asarray(W, dtype=np.float32)
    b = np.asarray(b, dtype=np.float32)
    assert input.shape == (B, D) and W.shape == (L, D) and b.shape == (L, D)

    with_bias = bool(np.any(b))
    nc = _get_nc(with_bias)

    wt = np.ascontiguousarray(W.T)  # [D, L]
    in_maps = []
    for i in range(NCORES):
        m = {"x": input[i * R : (i + 1) * R], "wt": wt}
        if with_bias:
            # B_i = sum_{j<i} b_j ; beta_i = B_i . w_i ; B_L = sum_j b_j
            Bpre = np.concatenate(
                [np.zeros((1, D), np.float32), np.cumsum(b, axis=0)[:-1]], axis=0
            )
            m["bl"] = b.sum(axis=0, dtype=np.float32).reshape(1, D)
            m["beta"] = np.einsum("ld,ld->l", Bpre, W).astype(np.float32).reshape(1, L)
        in_maps.append(m)

    res = run_bass_kernel_spmd(
        nc, in_maps, core_ids=list(range(NCORES)), **run_kwargs
    )
    out = np.concatenate([res.results[i]["y"] for i in range(NCORES)], axis=0)
    if run_kwargs:
        return out, res
    return out



# revision 24
# speedup vs baseline: 1.2394x; 1.2394x over previous
"""Trainium2 Bass kernel for nn_CrossNetwork (DCN-v1 cross network).

Math: reference computes x_{i+1} = input * (x_i . w_i) + x_i + b_i, L=6 layers.
Writing x_i = input * c_i + B_i with B_i = sum_{j<i} b_j (a constant row
vector) and c_i a per-row scalar, the recursion collapses to
    u_i    = input . w_i                     (per row, one tall-skinny matmul)
    beta_i = B_i . w_i                       (host-computed constants)
    c_{i+1} = c_i * (1 + u_i) + beta_i ; c_0 = 1
    out    = input * c_L + B_L
For the b == 0 case this is out = input * prod_i(1 + u_i).

Device work per core (2048 rows): load x once, PE-transpose 128x128 blocks
(f32r, 1.5 cyc/row), f32r matmul against W^T accumulating U^T[6, rows]
(1 cyc/row), DVE product-reduce to c, DVE per-partition-scalar multiply,
store. HBM traffic 16MB/core => memory-bound, roofline ~47us at 358GB/s.

Perf structure vs the original baseline (70.9us -> ~58-60us measured):
  - bf16 matmul inputs (fp32 matmul is 4 cyc/row on PE, bf16 is 1); the
    cast rides the ACT PSUM->SBUF copy, W cast once at startup. Dot
    products still accumulate in fp32 PSUM; global rel err ~5e-3 vs the
    2e-2 gate.
  - transpose(k+1) emitted before matmul(k) so the PE never stalls on the
    ACT PSUM->SBUF copy of transpose k.
  - DMA batched: 2x4MB loads, 4x2MB stores (STORE_GRP=1) instead of 32
    per-chunk 512KB transfers; measured best on HW.
  - c = prod(1+u_i) via a single DVE mult-reduce off PSUM.
"""

import os

import numpy as np

import concourse.bass as bass
import concourse.mybir as mybir
import concourse.tile as tile
from concourse.bass_utils import run_bass_kernel_spmd
from concourse.masks import make_identity
from concourse.vector_clock import ScopedClock

F32 = mybir.dt.float32
BF16 = mybir.dt.bfloat16

B, D, L = 16384, 1024, 6
NCORES = 8
R = B // NCORES  # rows per core
P = 128
NCH = R // P  # chunks of 128 rows per core
KB = D // P  # 128-wide k blocks
GRP = 4  # chunks per psum accumulation group (matmul N = 512, PSUM bank cap)
NG = NCH // GRP

# --- tuning knobs (env-overridable for A/B; defaults are the shipped config)
MM_BF16 = os.environ.get("K_MM_BF16", "1") == "1"
PIECES = int(os.environ.get("K_PIECES", "2"))  # DMA pieces per direction
LOAD_ENG = os.environ.get("K_LOAD_ENG", "sync")
STORE_ENG = os.environ.get("K_STORE_ENG", "sync")
# full | dma | load | load2 | store | dma_nodep
MODE = os.environ.get("K_MODE", "full")
# store granularity in units of GRP-chunk groups (1 => 2MB stores)
STORE_GRP = int(os.environ.get("K_STORE_GRP", "1"))
# alternate HWDGE rings per transfer: none | stores | loads | both
RING_SPLIT = os.environ.get("K_RING_SPLIT", "none")
GPC = NCH // PIECES  # chunks per DMA piece
assert NCH % PIECES == 0 and GPC % GRP == 0
assert (GPC // GRP) % STORE_GRP == 0  # stores must not span piece tiles


def _patch_tile_drain():
    """This walrus build rejects >1 sem wait on a CTRL (Drain) instruction.

    Tile's kernel-tail drain waits on every sem domain at once; split it into
    chained single-wait drains.
    """
    if getattr(tile.TileContext, "_drain_patched", False):
        return

    def _drain_and_barrier(self, tick_clock, wait_clock):
        gc = tick_clock.global_clock
        entries = [(proc, t) for proc, t in enumerate(gc) if t > 0]
        if not entries:
            self.nc.sync.drain()
        for proc, t in entries:
            sub = ScopedClock()
            sub.require_at_least(None, proc, t)
            drain_inst = self.nc.sync.drain()
            wait_clock.add_sem_waits(drain_inst.ins, sub)

        self.nc.all_engine_barrier()
        assert self.sems is not None
        popped = self.nc._tile_sem_poison_stack.pop()
        assert popped is self._sem_poison
        self.nc.clear_and_free_semaphores(list(self.sems.allocated().values()))

    tile.TileContext._drain_and_barrier = _drain_and_barrier
    tile.TileContext._drain_patched = True


def _build(with_bias: bool, loop_n: int = 1):
    nc = bass.Bass("TRN2")
    x_d = nc.dram_tensor("x", [R, D], F32, kind="ExternalInput")
    wt_d = nc.dram_tensor("wt", [D, L], F32, kind="ExternalInput")
    if with_bias:
        bl_d = nc.dram_tensor("bl", [1, D], F32, kind="ExternalInput")
        beta_d = nc.dram_tensor("beta", [1, L], F32, kind="ExternalInput")
    y_d = nc.dram_tensor("y", [R, D], F32, kind="ExternalOutput")

    xv = x_d.rearrange("(p n) d -> p n d", p=P)  # [128, NCH, D]
    yv = y_d.rearrange("(p n) d -> p n d", p=P)
    wtv = wt_d.rearrange("(k p) s -> p k s", p=P)  # [128, KB, L]

    with tile.TileContext(nc) as tc:
        with (
            tc.tile_pool(name="consts", bufs=1) as consts,
            tc.tile_pool(name="xch", bufs=PIECES) as xpool,
            tc.tile_pool(name="xt", bufs=4) as xtpool,
            tc.tile_pool(name="small", bufs=2 * GRP) as small,
            tc.tile_pool(name="pxt", bufs=4, space="PSUM") as pxt,
            tc.tile_pool(name="pu", bufs=2, space="PSUM") as pu,
        ):
            ident = consts.tile([P, P], F32)
            make_identity(nc, ident)
            src_sb = None
            if MODE in ("store", "dma_nodep"):
                src_sb = []
                for h in range(PIECES):
                    t = consts.tile([P, GPC, D], F32, name=f"src{h}")
                    nc.vector.memset(t, 1.0)
                    src_sb.append(t)
            ident6 = consts.tile([L, L], F32)
            make_identity(nc, ident6)
            wt_sb = consts.tile([P, KB, L], F32)
            nc.sync.dma_start(out=wt_sb, in_=wtv)
            if MM_BF16:
                wt_bf = consts.tile([P, KB, L], BF16)
                nc.scalar.copy(wt_bf, wt_sb)
                wt_sb = wt_bf
            bl_sb = beta_sb = None
            if with_bias:
                bl_sb = consts.tile([P, D], F32)
                nc.sync.dma_start(
                    out=bl_sb,
                    in_=bass.AP(tensor=bl_d, offset=0, ap=[[0, P], [1, D]]),
                )
                beta_sb = consts.tile([P, L], F32)
                nc.sync.dma_start(
                    out=beta_sb,
                    in_=bass.AP(tensor=beta_d, offset=0, ap=[[0, P], [1, L]]),
                )

            import contextlib
            loop_cm = (
                tc.For_i(0, loop_n, 1) if loop_n > 1 else contextlib.nullcontext()
            )
            with loop_cm:
                _body(nc, tc, xpool, xtpool, small, pxt, pu, ident, ident6,
                      wt_sb, bl_sb, beta_sb, xv, yv, with_bias, src_sb)
    return nc


def _body(nc, tc, xpool, xtpool, small, pxt, pu, ident, ident6,
          wt_sb, bl_sb, beta_sb, xv, yv, with_bias, src_sb=None):
    load_eng = getattr(nc, LOAD_ENG)
    store_eng = getattr(nc, STORE_ENG)

    if MODE in ("store", "dma_nodep"):
        # stores sourced from preset const tiles — no dependency coupling
        if MODE == "dma_nodep":
            for h in range(PIECES):
                t = xpool.tile([P, GPC, D], F32, tag="xch")
                load_eng.dma_start(
                    out=t, in_=xv[:, h * GPC : (h + 1) * GPC, :]
                )
        for h in range(PIECES):
            store_eng.dma_start(
                out=yv[:, h * GPC : (h + 1) * GPC, :], in_=src_sb[h]
            )
            if MODE == "store":
                store_eng.dma_start(
                    out=yv[:, h * GPC : (h + 1) * GPC, :], in_=src_sb[h]
                )
        return

    # big-piece loads: PIECES tiles of [128, GPC, D]
    x_sb = []
    for h in range(PIECES):
        t = xpool.tile([P, GPC, D], F32, tag="xch")
        eng = load_eng
        if MODE == "load2" or RING_SPLIT in ("loads", "both"):
            eng = (nc.sync, nc.scalar)[h % 2]
        eng.dma_start(out=t, in_=xv[:, h * GPC : (h + 1) * GPC, :])
        x_sb.append(t)

    if MODE in ("load", "load2"):
        # 16MB of pure reads (two passes over the 8MB input)
        for h in range(PIECES):
            eng = load_eng if MODE != "load2" else (nc.sync, nc.scalar)[h % 2]
            eng.dma_start(out=x_sb[h], in_=xv[:, h * GPC : (h + 1) * GPC, :])
        return

    if MODE == "dma":
        for h in range(PIECES):
            store_eng.dma_start(
                out=yv[:, h * GPC : (h + 1) * GPC, :], in_=x_sb[h]
            )
        return

    def xch(n):
        # chunk n as a [128, D] view into its piece tile
        return x_sb[n // GPC][:, n % GPC, :]

    # software pipeline: emit transposes for (g,k) one step ahead of the
    # matmul consuming (g,k-1)'s copied tile, so PE never waits on ACT.
    flat = [(g, k) for g in range(NG) for k in range(KB)]
    xt_tiles = {}
    xt_dt = BF16 if MM_BF16 else F32

    def emit_transpose_copy(g, k):
        pxt_t = pxt.tile([P, GRP * P], F32, tag="pxt")
        for j in range(GRP):
            src = xch(g * GRP + j)[:, k * P : (k + 1) * P]
            nc.tensor.transpose(pxt_t[:, j * P : (j + 1) * P], src, ident)
        xt_t = xtpool.tile([P, GRP * P], xt_dt, tag="xt")
        nc.scalar.copy(xt_t, pxt_t)
        xt_tiles[(g, k)] = xt_t

    ut_tiles = {}

    def emit_matmul(g, k):
        if k == 0:
            ut_tiles[g] = pu.tile([L, GRP * P], F32, tag="u", name=f"ut{g}")
        lhs = wt_sb[:, k, :]
        rhs = xt_tiles.pop((g, k))[:]
        nc.tensor.matmul(
            ut_tiles[g][:], lhs, rhs, start=(k == 0), stop=(k == KB - 1)
        )

    def emit_group_tail(g):
        # 1 + U^T on DVE while copying PSUM->SBUF, transpose [6,128] blocks
        # back to row-major [128,6], product-reduce to c, scale rows.
        ut_ps = ut_tiles.pop(g)
        u1t_t = xtpool.tile([L, GRP * P], F32, tag="u1t")
        nc.vector.tensor_scalar_add(u1t_t, ut_ps, 1.0)
        uj_ps = pu.tile([P, GRP, L], F32, tag="uj", name=f"uj{g}")
        for j in range(GRP):
            nc.tensor.transpose(
                uj_ps[:, j, :], u1t_t[:, j * P : (j + 1) * P], ident6
            )
        for j in range(GRP):
            n = g * GRP + j
            if not with_bias:
                # c = prod over the 6 (1+u_i): one mult-reduce off PSUM
                c_t = small.tile([P, 1], F32, tag="c")
                nc.vector.tensor_reduce(
                    c_t,
                    uj_ps[:, j, :],
                    axis=mybir.AxisListType.X,
                    op=mybir.AluOpType.mult,
                )
                nc.vector.tensor_scalar_mul(xch(n), xch(n), c_t)
            else:
                u1_t = small.tile([P, L], F32, tag="u1")
                nc.vector.tensor_copy(u1_t, uj_ps[:, j, :])
                c_t = small.tile([P, 1], F32, tag="c")
                nc.vector.memset(c_t, 1.0)
                for i in range(L):
                    # c = c * (1 + u_i) + beta_i
                    nc.vector.scalar_tensor_tensor(
                        out=c_t,
                        in0=c_t,
                        scalar=u1_t[:, i : i + 1],
                        in1=beta_sb[:, i : i + 1],
                        op0=mybir.AluOpType.mult,
                        op1=mybir.AluOpType.add,
                    )
                # out = x * c + B_L
                nc.vector.scalar_tensor_tensor(
                    out=xch(n),
                    in0=xch(n),
                    scalar=c_t,
                    in1=bl_sb,
                    op0=mybir.AluOpType.mult,
                    op1=mybir.AluOpType.add,
                )
        # store once STORE_GRP groups are scaled
        if (g + 1) % STORE_GRP == 0:
            s = g // STORE_GRP  # store index
            n0 = (g + 1 - STORE_GRP) * GRP  # first chunk of this store
            n1 = (g + 1) * GRP
            h = n0 // GPC
            eng = store_eng
            if RING_SPLIT in ("stores", "both"):
                eng = (nc.sync, nc.scalar)[s % 2]
            eng.dma_start(
                out=yv[:, n0:n1, :],
                in_=x_sb[h][:, n0 - h * GPC : n1 - h * GPC, :],
            )

    # pipelined emission
    emit_transpose_copy(*flat[0])
    for i, (g, k) in enumerate(flat):
        if i + 1 < len(flat):
            emit_transpose_copy(*flat[i + 1])
        emit_matmul(g, k)
        if k == KB - 1:
            emit_group_tail(g)


def _split_multi_waits(nc):
    """This walrus build allows only one sem wait on several instruction
    structs (e.g. self-loading Matmult). Move extra waits onto preceding
    same-engine NOPs; engine FIFO order makes this equivalent."""
    n = 0
    for fn in nc.m.functions:
        for bb in fn.blocks:
            out = []
            for inst in bb.instructions:
                si = inst.sync_info
                if si is not None and si.on_wait and len(si.on_wait) > 1:
                    waits = list(si.on_wait)
                    for w in waits[:-1]:
                        n += 1
                        out.append(
                            mybir.InstNoOp(
                                name=f"nopw-{n}-{inst.name}",
                                engine=inst.engine,
                                sync_info=mybir.SyncInfo(
                                    on_wait=[w], on_update=[]
                                ),
                                bass_nofuse=True,
                            )
                        )
                    inst.sync_info = mybir.SyncInfo(
                        on_wait=[waits[-1]], on_update=list(si.on_update)
                    )
                out.append(inst)
            bb.instructions = out


_CACHE = {}


def _get_nc(with_bias: bool, loop_n: int = 1):
    key = (with_bias, loop_n, MM_BF16, PIECES, LOAD_ENG, STORE_ENG, MODE,
           STORE_GRP, RING_SPLIT)
    if key not in _CACHE:
        _patch_tile_drain()
        nc = _build(with_bias, loop_n)
        _split_multi_waits(nc)
        _CACHE[key] = nc
    return _CACHE[key]


def kernel(input, W, b, **run_kwargs):
    input = np.ascontiguousarray(np.asarray(input, dtype=np.float32))
    W = np.asarray(W, dtype=np.float32)
    b = np.asarray(b, dtype=np.float32)
    assert input.shape == (B, D) and W.shape == (L, D) and b.shape == (L, D)

    with_bias = bool(np.any(b))
    nc = _get_nc(with_bias)

    wt = np.ascontiguousarray(W.T)  # [D, L]
    in_maps = []
    for i in range(NCORES):
        m = {"x": input[i * R : (i + 1) * R], "wt": wt}
        if with_bias:
            # B_i = sum_{j<i} b_j ; beta_i = B_i . w_i ; B_L = sum_j b_j
            Bpre = np.concatenate(
                [np.zeros((1, D), np.float32), np.cumsum(b, axis=0)[:-1]], axis=0
            )
            m["bl"] = b.sum(axis=0, dtype=np.float32).reshape(1, D)
            m["beta"] = np.einsum("ld,ld->l", Bpre, W).astype(np.float32).reshape(1, L)
        in_maps.append(m)

    res = run_bass_kernel_spmd(
        nc, in_maps, core_ids=list(range(NCORES)), **run_kwargs
    )
    out = np.concatenate([res.results[i]["y"] for i in range(NCORES)], axis=0)
    if run_kwargs:
        return out, res
    return out


# revision 29
# speedup vs baseline: 1.3890x; 1.1207x over previous
"""Trainium2 Bass kernel for nn_CrossNetwork (DCN-v1 cross network).

Math: reference computes x_{i+1} = input * (x_i . w_i) + x_i + b_i, L=6 layers.
Writing x_i = input * c_i + B_i with B_i = sum_{j<i} b_j (a constant row
vector) and c_i a per-row scalar, the recursion collapses to
    u_i    = input . w_i                     (per row, one tall-skinny matmul)
    beta_i = B_i . w_i                       (host-computed constants)
    c_{i+1} = c_i * (1 + u_i) + beta_i ; c_0 = 1
    out    = input * c_L + B_L
For the b == 0 case this is out = input * prod_i(1 + u_i).

Device work per core (2048 rows): load x once, PE-transpose 128x128 blocks
(f32r, 1.5 cyc/row), f32r matmul against W^T accumulating U^T[6, rows]
(1 cyc/row), DVE product-reduce to c, DVE per-partition-scalar multiply,
store. HBM traffic 16MB/core => memory-bound, roofline ~47us at 358GB/s.

Perf structure vs the original baseline (70.9us -> ~58-60us measured):
  - bf16 matmul inputs (fp32 matmul is 4 cyc/row on PE, bf16 is 1); the
    cast rides the ACT PSUM->SBUF copy, W cast once at startup. Dot
    products still accumulate in fp32 PSUM; global rel err ~5e-3 vs the
    2e-2 gate.
  - transpose(k+1) emitted before matmul(k) so the PE never stalls on the
    ACT PSUM->SBUF copy of transpose k.
  - DMA batched: 4x2MB loads (PIECES=4), 4x2MB stores (STORE_GRP=1)
    instead of 32 per-chunk 512KB transfers; measured best on HW
    (44-56us vs 58-60 for 2x4MB loads).
  - c = prod(1+u_i) via a single DVE mult-reduce off PSUM.
"""

import os

import numpy as np

import concourse.bass as bass
import concourse.mybir as mybir
import concourse.tile as tile
from concourse.bass_utils import run_bass_kernel_spmd
from concourse.masks import make_identity
from concourse.vector_clock import ScopedClock

F32 = mybir.dt.float32
BF16 = mybir.dt.bfloat16

B, D, L = 16384, 1024, 6
NCORES = 8
R = B // NCORES  # rows per core
P = 128
NCH = R // P  # chunks of 128 rows per core
KB = D // P  # 128-wide k blocks
GRP = 4  # chunks per psum accumulation group (matmul N = 512, PSUM bank cap)
NG = NCH // GRP

# --- tuning knobs (env-overridable for A/B; defaults are the shipped config)
MM_BF16 = os.environ.get("K_MM_BF16", "1") == "1"
PIECES = int(os.environ.get("K_PIECES", "4"))  # DMA pieces per direction
LOAD_ENG = os.environ.get("K_LOAD_ENG", "sync")
STORE_ENG = os.environ.get("K_STORE_ENG", "sync")
# full | dma | load | load2 | store | dma_nodep
MODE = os.environ.get("K_MODE", "full")
# store granularity in units of GRP-chunk groups (1 => 2MB stores)
STORE_GRP = int(os.environ.get("K_STORE_GRP", "1"))
# alternate HWDGE rings per transfer: none | stores | loads | both
RING_SPLIT = os.environ.get("K_RING_SPLIT", "none")
# send odd-chunk row-scales to ACT instead of DVE
SCALE_SPLIT = os.environ.get("K_SCALE_SPLIT", "0") == "1"
GPC = NCH // PIECES  # chunks per DMA piece
assert NCH % PIECES == 0 and GPC % GRP == 0
assert (GPC // GRP) % STORE_GRP == 0  # stores must not span piece tiles


def _patch_tile_drain():
    """This walrus build rejects >1 sem wait on a CTRL (Drain) instruction.

    Tile's kernel-tail drain waits on every sem domain at once; split it into
    chained single-wait drains.
    """
    if getattr(tile.TileContext, "_drain_patched", False):
        return

    def _drain_and_barrier(self, tick_clock, wait_clock):
        gc = tick_clock.global_clock
        entries = [(proc, t) for proc, t in enumerate(gc) if t > 0]
        if not entries:
            self.nc.sync.drain()
        for proc, t in entries:
            sub = ScopedClock()
            sub.require_at_least(None, proc, t)
            drain_inst = self.nc.sync.drain()
            wait_clock.add_sem_waits(drain_inst.ins, sub)

        self.nc.all_engine_barrier()
        assert self.sems is not None
        popped = self.nc._tile_sem_poison_stack.pop()
        assert popped is self._sem_poison
        self.nc.clear_and_free_semaphores(list(self.sems.allocated().values()))

    tile.TileContext._drain_and_barrier = _drain_and_barrier
    tile.TileContext._drain_patched = True


def _build(with_bias: bool, loop_n: int = 1):
    nc = bass.Bass("TRN2")
    x_d = nc.dram_tensor("x", [R, D], F32, kind="ExternalInput")
    wt_d = nc.dram_tensor("wt", [D, L], F32, kind="ExternalInput")
    if with_bias:
        bl_d = nc.dram_tensor("bl", [1, D], F32, kind="ExternalInput")
        beta_d = nc.dram_tensor("beta", [1, L], F32, kind="ExternalInput")
    y_d = nc.dram_tensor("y", [R, D], F32, kind="ExternalOutput")

    xv = x_d.rearrange("(p n) d -> p n d", p=P)  # [128, NCH, D]
    yv = y_d.rearrange("(p n) d -> p n d", p=P)
    wtv = wt_d.rearrange("(k p) s -> p k s", p=P)  # [128, KB, L]

    with tile.TileContext(nc) as tc:
        with (
            tc.tile_pool(name="consts", bufs=1) as consts,
            tc.tile_pool(name="xch", bufs=PIECES) as xpool,
            tc.tile_pool(name="xt", bufs=4) as xtpool,
            tc.tile_pool(name="small", bufs=2 * GRP) as small,
            tc.tile_pool(name="pxt", bufs=4, space="PSUM") as pxt,
            tc.tile_pool(name="pu", bufs=2, space="PSUM") as pu,
        ):
            ident = consts.tile([P, P], F32)
            make_identity(nc, ident)
            src_sb = None
            if MODE in ("store", "dma_nodep"):
                src_sb = []
                for h in range(PIECES):
                    t = consts.tile([P, GPC, D], F32, name=f"src{h}")
                    nc.vector.memset(t, 1.0)
                    src_sb.append(t)
            ident6 = consts.tile([L, L], F32)
            make_identity(nc, ident6)
            wt_sb = consts.tile([P, KB, L], F32)
            nc.sync.dma_start(out=wt_sb, in_=wtv)
            if MM_BF16:
                wt_bf = consts.tile([P, KB, L], BF16)
                nc.scalar.copy(wt_bf, wt_sb)
                wt_sb = wt_bf
            bl_sb = beta_sb = None
            if with_bias:
                bl_sb = consts.tile([P, D], F32)
                nc.sync.dma_start(
                    out=bl_sb,
                    in_=bass.AP(tensor=bl_d, offset=0, ap=[[0, P], [1, D]]),
                )
                beta_sb = consts.tile([P, L], F32)
                nc.sync.dma_start(
                    out=beta_sb,
                    in_=bass.AP(tensor=beta_d, offset=0, ap=[[0, P], [1, L]]),
                )

            import contextlib
            loop_cm = (
                tc.For_i(0, loop_n, 1) if loop_n > 1 else contextlib.nullcontext()
            )
            with loop_cm:
                _body(nc, tc, xpool, xtpool, small, pxt, pu, ident, ident6,
                      wt_sb, bl_sb, beta_sb, xv, yv, with_bias, src_sb)
    return nc


def _body(nc, tc, xpool, xtpool, small, pxt, pu, ident, ident6,
          wt_sb, bl_sb, beta_sb, xv, yv, with_bias, src_sb=None):
    load_eng = getattr(nc, LOAD_ENG)
    store_eng = getattr(nc, STORE_ENG)

    if MODE in ("store", "dma_nodep"):
        # stores sourced from preset const tiles — no dependency coupling
        if MODE == "dma_nodep":
            for h in range(PIECES):
                t = xpool.tile([P, GPC, D], F32, tag="xch")
                load_eng.dma_start(
                    out=t, in_=xv[:, h * GPC : (h + 1) * GPC, :]
                )
        for h in range(PIECES):
            store_eng.dma_start(
                out=yv[:, h * GPC : (h + 1) * GPC, :], in_=src_sb[h]
            )
            if MODE == "store":
                store_eng.dma_start(
                    out=yv[:, h * GPC : (h + 1) * GPC, :], in_=src_sb[h]
                )
        return

    # big-piece loads: PIECES tiles of [128, GPC, D]
    x_sb = []
    for h in range(PIECES):
        t = xpool.tile([P, GPC, D], F32, tag="xch")
        eng = load_eng
        if MODE == "load2" or RING_SPLIT in ("loads", "both"):
            eng = (nc.sync, nc.scalar)[h % 2]
        eng.dma_start(out=t, in_=xv[:, h * GPC : (h + 1) * GPC, :])
        x_sb.append(t)

    if MODE in ("load", "load2"):
        # 16MB of pure reads (two passes over the 8MB input)
        for h in range(PIECES):
            eng = load_eng if MODE != "load2" else (nc.sync, nc.scalar)[h % 2]
            eng.dma_start(out=x_sb[h], in_=xv[:, h * GPC : (h + 1) * GPC, :])
        return

    if MODE == "dma":
        for h in range(PIECES):
            store_eng.dma_start(
                out=yv[:, h * GPC : (h + 1) * GPC, :], in_=x_sb[h]
            )
        return

    def xch(n):
        # chunk n as a [128, D] view into its piece tile
        return x_sb[n // GPC][:, n % GPC, :]

    # software pipeline: emit transposes for (g,k) one step ahead of the
    # matmul consuming (g,k-1)'s copied tile, so PE never waits on ACT.
    flat = [(g, k) for g in range(NG) for k in range(KB)]
    xt_tiles = {}
    xt_dt = BF16 if MM_BF16 else F32

    def emit_transpose_copy(g, k):
        pxt_t = pxt.tile([P, GRP * P], F32, tag="pxt")
        for j in range(GRP):
            src = xch(g * GRP + j)[:, k * P : (k + 1) * P]
            nc.tensor.transpose(pxt_t[:, j * P : (j + 1) * P], src, ident)
        xt_t = xtpool.tile([P, GRP * P], xt_dt, tag="xt")
        nc.scalar.copy(xt_t, pxt_t)
        xt_tiles[(g, k)] = xt_t

    ut_tiles = {}

    def emit_matmul(g, k):
        if k == 0:
            ut_tiles[g] = pu.tile([L, GRP * P], F32, tag="u", name=f"ut{g}")
        lhs = wt_sb[:, k, :]
        rhs = xt_tiles.pop((g, k))[:]
        nc.tensor.matmul(
            ut_tiles[g][:], lhs, rhs, start=(k == 0), stop=(k == KB - 1)
        )

    def emit_group_tail(g):
        # 1 + U^T on DVE while copying PSUM->SBUF, transpose [6,128] blocks
        # back to row-major [128,6], product-reduce to c, scale rows.
        ut_ps = ut_tiles.pop(g)
        u1t_t = xtpool.tile([L, GRP * P], F32, tag="u1t")
        nc.vector.tensor_scalar_add(u1t_t, ut_ps, 1.0)
        uj_ps = pu.tile([P, GRP, L], F32, tag="uj", name=f"uj{g}")
        for j in range(GRP):
            nc.tensor.transpose(
                uj_ps[:, j, :], u1t_t[:, j * P : (j + 1) * P], ident6
            )
        for j in range(GRP):
            n = g * GRP + j
            if not with_bias:
                # c = prod over the 6 (1+u_i): one mult-reduce off PSUM
                c_t = small.tile([P, 1], F32, tag="c")
                nc.vector.tensor_reduce(
                    c_t,
                    uj_ps[:, j, :],
                    axis=mybir.AxisListType.X,
                    op=mybir.AluOpType.mult,
                )
                if SCALE_SPLIT and j % 2 == 1:
                    nc.scalar.activation(
                        xch(n),
                        xch(n),
                        mybir.ActivationFunctionType.Copy,
                        scale=c_t[:],
                    )
                else:
                    nc.vector.tensor_scalar_mul(xch(n), xch(n), c_t)
            else:
                u1_t = small.tile([P, L], F32, tag="u1")
                nc.vector.tensor_copy(u1_t, uj_ps[:, j, :])
                c_t = small.tile([P, 1], F32, tag="c")
                nc.vector.memset(c_t, 1.0)
                for i in range(L):
                    # c = c * (1 + u_i) + beta_i
                    nc.vector.scalar_tensor_tensor(
                        out=c_t,
                        in0=c_t,
                        scalar=u1_t[:, i : i + 1],
                        in1=beta_sb[:, i : i + 1],
                        op0=mybir.AluOpType.mult,
                        op1=mybir.AluOpType.add,
                    )
                # out = x * c + B_L
                nc.vector.scalar_tensor_tensor(
                    out=xch(n),
                    in0=xch(n),
                    scalar=c_t,
                    in1=bl_sb,
                    op0=mybir.AluOpType.mult,
                    op1=mybir.AluOpType.add,
                )
        # store once STORE_GRP groups are scaled
        if (g + 1) % STORE_GRP == 0:
            s = g // STORE_GRP  # store index
            n0 = (g + 1 - STORE_GRP) * GRP  # first chunk of this store
            n1 = (g + 1) * GRP
            h = n0 // GPC
            eng = store_eng
            if RING_SPLIT in ("stores", "both"):
                eng = (nc.sync, nc.scalar)[s % 2]
            eng.dma_start(
                out=yv[:, n0:n1, :],
                in_=x_sb[h][:, n0 - h * GPC : n1 - h * GPC, :],
            )

    # pipelined emission
    emit_transpose_copy(*flat[0])
    for i, (g, k) in enumerate(flat):
        if i + 1 < len(flat):
            emit_transpose_copy(*flat[i + 1])
        emit_matmul(g, k)
        if k == KB - 1:
            emit_group_tail(g)


def _split_multi_waits(nc):
    """This walrus build allows only one sem wait on several instruction
    structs (e.g. self-loading Matmult). Move extra waits onto preceding
    same-engine NOPs; engine FIFO order makes this equivalent."""
    n = 0
    for fn in nc.m.functions:
        for bb in fn.blocks:
            out = []
            for inst in bb.instructions:
                si = inst.sync_info
                if si is not None and si.on_wait and len(si.on_wait) > 1:
                    waits = list(si.on_wait)
                    for w in waits[:-1]:
                        n += 1
                        out.append(
                            mybir.InstNoOp(
                                name=f"nopw-{n}-{inst.name}",
                                engine=inst.engine,
                                sync_info=mybir.SyncInfo(
                                    on_wait=[w], on_update=[]
                                ),
                                bass_nofuse=True,
                            )
                        )
                    inst.sync_info = mybir.SyncInfo(
                        on_wait=[waits[-1]], on_update=list(si.on_update)
                    )
                out.append(inst)
            bb.instructions = out


_CACHE = {}


def _get_nc(with_bias: bool, loop_n: int = 1):
    key = (with_bias, loop_n, MM_BF16, PIECES, LOAD_ENG, STORE_ENG, MODE,
           STORE_GRP, RING_SPLIT, SCALE_SPLIT)
    if key not in _CACHE:
        _patch_tile_drain()
        nc = _build(with_bias, loop_n)
        _split_multi_waits(nc)
        _CACHE[key] = nc
    return _CACHE[key]


def kernel(input, W, b, **run_kwargs):
    input = np.ascontiguousarray(np.asarray(input, dtype=np.float32))
    W = np.asarray(W, dtype=np.float32)
    b = np.asarray(b, dtype=np.float32)
    assert input.shape == (B, D) and W.shape == (L, D) and b.shape == (L, D)

    with_bias = bool(np.any(b))
    nc = _get_nc(with_bias)

    wt = np.ascontiguousarray(W.T)  # [D, L]
    in_maps = []
    for i in range(NCORES):
        m = {"x": input[i * R : (i + 1) * R], "wt": wt}
        if with_bias:
            # B_i = sum_{j<i} b_j ; beta_i = B_i . w_i ; B_L = sum_j b_j
            Bpre = np.concatenate(
                [np.zeros((1, D), np.float32), np.cumsum(b, axis=0)[:-1]], axis=0
            )
            m["bl"] = b.sum(axis=0, dtype=np.float32).reshape(1, D)
            m["beta"] = np.einsum("ld,ld->l", Bpre, W).astype(np.float32).reshape(1, L)
        in_maps.append(m)

    res = run_bass_kernel_spmd(
        nc, in_maps, core_ids=list(range(NCORES)), **run_kwargs
    )
    out = np.concatenate([res.results[i]["y"] for i in range(NCORES)], axis=0)
    if run_kwargs:
        return out, res
    return out


# revision 47
# speedup vs baseline: 1.4745x; 1.0615x over previous
"""Trainium2 Bass kernel for nn_CrossNetwork (DCN-v1 cross network).

Math: reference computes x_{i+1} = input * (x_i . w_i) + x_i + b_i, L=6 layers.
Writing x_i = input * c_i + B_i with B_i = sum_{j<i} b_j (a constant row
vector) and c_i a per-row scalar, the recursion collapses to
    u_i    = input . w_i                     (per row, one tall-skinny matmul)
    beta_i = B_i . w_i                       (host-computed constants)
    c_{i+1} = c_i * (1 + u_i) + beta_i ; c_0 = 1
    out    = input * c_L + B_L
For the b == 0 case this is out = input * prod_i(1 + u_i).

Device work per core (2048 rows): load x once, PE-transpose 128x128 blocks
(f32r, 1.5 cyc/row), f32r matmul against W^T accumulating U^T[6, rows]
(1 cyc/row), DVE product-reduce to c, DVE per-partition-scalar multiply,
store. HBM traffic 16MB/core => memory-bound, roofline ~47us at 358GB/s.

Perf structure vs the original baseline (70.9us -> ~58-60us measured):
  - bf16 matmul inputs (fp32 matmul is 4 cyc/row on PE, bf16 is 1); the
    cast rides the ACT PSUM->SBUF copy, W cast once at startup. Dot
    products still accumulate in fp32 PSUM; global rel err ~5e-3 vs the
    2e-2 gate.
  - transpose(k+1) emitted before matmul(k) so the PE never stalls on the
    ACT PSUM->SBUF copy of transpose k.
  - DMA batched: 4x2MB loads (PIECES=4), 4x2MB stores (STORE_GRP=1)
    instead of 32 per-chunk 512KB transfers; measured best on HW
    (44-56us vs 58-60 for 2x4MB loads).
  - c = prod(1+u_i) via a single DVE mult-reduce off PSUM.
"""

import os

import numpy as np

import concourse.bass as bass
import concourse.mybir as mybir
import concourse.tile as tile
from concourse.bass_utils import run_bass_kernel_spmd
from concourse.masks import make_identity
from concourse.vector_clock import ScopedClock

F32 = mybir.dt.float32
BF16 = mybir.dt.bfloat16

B, D, L = 16384, 1024, 6
NCORES = 8
R = B // NCORES  # rows per core
P = 128
NCH = R // P  # chunks of 128 rows per core
KB = D // P  # 128-wide k blocks
# chunks per psum accumulation group (matmul N = GRP*128; 4 => 512, the
# fp32 PSUM bank cap)
GRP = int(os.environ.get("K_GRP", "4"))
NG = NCH // GRP

# --- tuning knobs (env-overridable for A/B; defaults are the shipped config)
MM_BF16 = os.environ.get("K_MM_BF16", "1") == "1"
PIECES = int(os.environ.get("K_PIECES", "4"))  # DMA pieces per direction
LOAD_ENG = os.environ.get("K_LOAD_ENG", "sync")
STORE_ENG = os.environ.get("K_STORE_ENG", "sync")
# full | dma | load | load2 | store | dma_nodep
MODE = os.environ.get("K_MODE", "full")
# store granularity in units of GRP-chunk groups (1 => 2MB stores)
STORE_GRP = int(os.environ.get("K_STORE_GRP", "1"))
# alternate HWDGE rings per transfer: none | stores | loads | both
RING_SPLIT = os.environ.get("K_RING_SPLIT", "none")
# send odd-chunk row-scales to ACT instead of DVE
SCALE_SPLIT = os.environ.get("K_SCALE_SPLIT", "0") == "1"
# issue load piece h+2 after store h instead of all loads upfront, so the
# HWDGE ring interleaves reads and writes (requires GPC == GRP)
ISSUE_LAZY = os.environ.get("K_ISSUE_LAZY", "0") == "1"
# store y as bf16 (half the write traffic), upcast host-side; output
# quantization ~2e-3 rel err, inside the 2e-2 gate
OUT_BF16 = os.environ.get("K_OUT_BF16", "1") == "1"
GPC = NCH // PIECES  # chunks per DMA piece
assert NCH % PIECES == 0 and GPC % GRP == 0
assert (GPC // GRP) % STORE_GRP == 0  # stores must not span piece tiles


def _patch_tile_drain():
    """This walrus build rejects >1 sem wait on a CTRL (Drain) instruction.

    Tile's kernel-tail drain waits on every sem domain at once; split it into
    chained single-wait drains.
    """
    if getattr(tile.TileContext, "_drain_patched", False):
        return

    def _drain_and_barrier(self, tick_clock, wait_clock):
        gc = tick_clock.global_clock
        entries = [(proc, t) for proc, t in enumerate(gc) if t > 0]
        if not entries:
            self.nc.sync.drain()
        for proc, t in entries:
            sub = ScopedClock()
            sub.require_at_least(None, proc, t)
            drain_inst = self.nc.sync.drain()
            wait_clock.add_sem_waits(drain_inst.ins, sub)

        self.nc.all_engine_barrier()
        assert self.sems is not None
        popped = self.nc._tile_sem_poison_stack.pop()
        assert popped is self._sem_poison
        self.nc.clear_and_free_semaphores(list(self.sems.allocated().values()))

    tile.TileContext._drain_and_barrier = _drain_and_barrier
    tile.TileContext._drain_patched = True


def _build(with_bias: bool, loop_n: int = 1):
    nc = bass.Bass("TRN2")
    x_d = nc.dram_tensor("x", [R, D], F32, kind="ExternalInput")
    wt_d = nc.dram_tensor("wt", [D, L], F32, kind="ExternalInput")
    if with_bias:
        bl_d = nc.dram_tensor("bl", [1, D], F32, kind="ExternalInput")
        beta_d = nc.dram_tensor("beta", [1, L], F32, kind="ExternalInput")
    out_bf = OUT_BF16 and not with_bias and MODE == "full"
    y_d = nc.dram_tensor(
        "y", [R, D], BF16 if out_bf else F32, kind="ExternalOutput"
    )

    xv = x_d.rearrange("(p n) d -> p n d", p=P)  # [128, NCH, D]
    yv = y_d.rearrange("(p n) d -> p n d", p=P)
    wtv = wt_d.rearrange("(k p) s -> p k s", p=P)  # [128, KB, L]

    with tile.TileContext(nc) as tc:
        nwin = NCH // (STORE_GRP * GRP)
        with (
            tc.tile_pool(name="consts", bufs=1) as consts,
            tc.tile_pool(name="xch", bufs=PIECES) as xpool,
            tc.tile_pool(name="xt", bufs=4) as xtpool,
            tc.tile_pool(name="small", bufs=2 * GRP) as small,
            tc.tile_pool(name="ybf", bufs=min(nwin, 4)) as ypool,
            tc.tile_pool(name="pxt", bufs=4, space="PSUM") as pxt,
            tc.tile_pool(name="pu", bufs=2, space="PSUM") as pu,
        ):
            ident = consts.tile([P, P], F32)
            make_identity(nc, ident)
            src_sb = None
            if MODE in ("store", "dma_nodep"):
                src_sb = []
                for h in range(PIECES):
                    t = consts.tile([P, GPC, D], F32, name=f"src{h}")
                    nc.vector.memset(t, 1.0)
                    src_sb.append(t)
            ident6 = consts.tile([L, L], F32)
            make_identity(nc, ident6)
            wt_sb = consts.tile([P, KB, L], F32)
            nc.sync.dma_start(out=wt_sb, in_=wtv)
            if MM_BF16:
                wt_bf = consts.tile([P, KB, L], BF16)
                nc.scalar.copy(wt_bf, wt_sb)
                wt_sb = wt_bf
            bl_sb = beta_sb = None
            if with_bias:
                bl_sb = consts.tile([P, D], F32)
                nc.sync.dma_start(
                    out=bl_sb,
                    in_=bass.AP(tensor=bl_d, offset=0, ap=[[0, P], [1, D]]),
                )
                beta_sb = consts.tile([P, L], F32)
                nc.sync.dma_start(
                    out=beta_sb,
                    in_=bass.AP(tensor=beta_d, offset=0, ap=[[0, P], [1, L]]),
                )

            import contextlib
            loop_cm = (
                tc.For_i(0, loop_n, 1) if loop_n > 1 else contextlib.nullcontext()
            )
            with loop_cm:
                _body(nc, tc, xpool, xtpool, small, ypool, pxt, pu, ident,
                      ident6, wt_sb, bl_sb, beta_sb, xv, yv, with_bias,
                      src_sb, out_bf)
    return nc


def _body(nc, tc, xpool, xtpool, small, ypool, pxt, pu, ident, ident6,
          wt_sb, bl_sb, beta_sb, xv, yv, with_bias, src_sb=None,
          out_bf=False):
    load_eng = getattr(nc, LOAD_ENG)
    store_eng = getattr(nc, STORE_ENG)

    if MODE in ("store", "dma_nodep"):
        # stores sourced from preset const tiles — no dependency coupling
        if MODE == "dma_nodep":
            for h in range(PIECES):
                t = xpool.tile([P, GPC, D], F32, tag="xch")
                load_eng.dma_start(
                    out=t, in_=xv[:, h * GPC : (h + 1) * GPC, :]
                )
        for h in range(PIECES):
            store_eng.dma_start(
                out=yv[:, h * GPC : (h + 1) * GPC, :], in_=src_sb[h]
            )
            if MODE == "store":
                store_eng.dma_start(
                    out=yv[:, h * GPC : (h + 1) * GPC, :], in_=src_sb[h]
                )
        return

    # big-piece loads: PIECES tiles of [128, GPC, D]
    lazy = ISSUE_LAZY and GPC == GRP and MODE == "full"
    x_sb = [None] * PIECES

    def load_piece(h):
        t = xpool.tile([P, GPC, D], F32, tag="xch")
        eng = load_eng
        if MODE == "load2" or RING_SPLIT in ("loads", "both"):
            eng = (nc.sync, nc.scalar)[h % 2]
        eng.dma_start(out=t, in_=xv[:, h * GPC : (h + 1) * GPC, :])
        x_sb[h] = t

    for h in range(2 if lazy else PIECES):
        load_piece(h)

    if MODE in ("load", "load2"):
        # 16MB of pure reads (two passes over the 8MB input)
        for h in range(PIECES):
            eng = load_eng if MODE != "load2" else (nc.sync, nc.scalar)[h % 2]
            eng.dma_start(out=x_sb[h], in_=xv[:, h * GPC : (h + 1) * GPC, :])
        return

    if MODE == "dma":
        for h in range(PIECES):
            store_eng.dma_start(
                out=yv[:, h * GPC : (h + 1) * GPC, :], in_=x_sb[h]
            )
        return

    def xch(n):
        # chunk n as a [128, D] view into its piece tile
        return x_sb[n // GPC][:, n % GPC, :]

    # software pipeline: emit transposes for (g,k) one step ahead of the
    # matmul consuming (g,k-1)'s copied tile, so PE never waits on ACT.
    flat = [(g, k) for g in range(NG) for k in range(KB)]
    xt_tiles = {}
    xt_dt = BF16 if MM_BF16 else F32

    def emit_transpose_copy(g, k):
        pxt_t = pxt.tile([P, GRP * P], F32, tag="pxt")
        for j in range(GRP):
            src = xch(g * GRP + j)[:, k * P : (k + 1) * P]
            nc.tensor.transpose(pxt_t[:, j * P : (j + 1) * P], src, ident)
        xt_t = xtpool.tile([P, GRP * P], xt_dt, tag="xt")
        nc.scalar.copy(xt_t, pxt_t)
        xt_tiles[(g, k)] = xt_t

    ut_tiles = {}

    def emit_matmul(g, k):
        if k == 0:
            ut_tiles[g] = pu.tile([L, GRP * P], F32, tag="u", name=f"ut{g}")
        lhs = wt_sb[:, k, :]
        rhs = xt_tiles.pop((g, k))[:]
        nc.tensor.matmul(
            ut_tiles[g][:], lhs, rhs, start=(k == 0), stop=(k == KB - 1)
        )

    ywin = {}

    def emit_group_tail(g):
        # 1 + U^T on DVE while copying PSUM->SBUF, transpose [6,128] blocks
        # back to row-major [128,6], product-reduce to c, scale rows.
        w, slot = g // STORE_GRP, g % STORE_GRP
        if out_bf and slot == 0:
            ywin[w] = ypool.tile(
                [P, STORE_GRP * GRP, D], BF16, tag="ybf", name=f"yw{w}"
            )
        ut_ps = ut_tiles.pop(g)
        u1t_t = xtpool.tile([L, GRP * P], F32, tag="u1t")
        nc.vector.tensor_scalar_add(u1t_t, ut_ps, 1.0)
        uj_ps = pu.tile([P, GRP, L], F32, tag="uj", name=f"uj{g}")
        for j in range(GRP):
            nc.tensor.transpose(
                uj_ps[:, j, :], u1t_t[:, j * P : (j + 1) * P], ident6
            )
        for j in range(GRP):
            n = g * GRP + j
            if not with_bias:
                # c = prod over the 6 (1+u_i): one mult-reduce off PSUM
                c_t = small.tile([P, 1], F32, tag="c")
                nc.vector.tensor_reduce(
                    c_t,
                    uj_ps[:, j, :],
                    axis=mybir.AxisListType.X,
                    op=mybir.AluOpType.mult,
                )
                y_ap = ywin[w][:, slot * GRP + j, :] if out_bf else xch(n)
                if SCALE_SPLIT and j % 2 == 1:
                    nc.scalar.activation(
                        y_ap,
                        xch(n),
                        mybir.ActivationFunctionType.Copy,
                        scale=c_t[:],
                    )
                else:
                    nc.vector.tensor_scalar_mul(y_ap, xch(n), c_t)
            else:
                u1_t = small.tile([P, L], F32, tag="u1")
                nc.vector.tensor_copy(u1_t, uj_ps[:, j, :])
                c_t = small.tile([P, 1], F32, tag="c")
                nc.vector.memset(c_t, 1.0)
                for i in range(L):
                    # c = c * (1 + u_i) + beta_i
                    nc.vector.scalar_tensor_tensor(
                        out=c_t,
                        in0=c_t,
                        scalar=u1_t[:, i : i + 1],
                        in1=beta_sb[:, i : i + 1],
                        op0=mybir.AluOpType.mult,
                        op1=mybir.AluOpType.add,
                    )
                # out = x * c + B_L
                nc.vector.scalar_tensor_tensor(
                    out=xch(n),
                    in0=xch(n),
                    scalar=c_t,
                    in1=bl_sb,
                    op0=mybir.AluOpType.mult,
                    op1=mybir.AluOpType.add,
                )
        # store once STORE_GRP groups are scaled
        if (g + 1) % STORE_GRP == 0:
            s = g // STORE_GRP  # store index
            n0 = (g + 1 - STORE_GRP) * GRP  # first chunk of this store
            n1 = (g + 1) * GRP
            h = n0 // GPC
            eng = store_eng
            if RING_SPLIT in ("stores", "both"):
                eng = (nc.sync, nc.scalar)[s % 2]
            src = (
                ywin.pop(s)
                if out_bf
                else x_sb[h][:, n0 - h * GPC : n1 - h * GPC, :]
            )
            eng.dma_start(out=yv[:, n0:n1, :], in_=src)
            if lazy and s + 2 < PIECES:
                load_piece(s + 2)

    # pipelined emission
    emit_transpose_copy(*flat[0])
    for i, (g, k) in enumerate(flat):
        if i + 1 < len(flat):
            emit_transpose_copy(*flat[i + 1])
        emit_matmul(g, k)
        if k == KB - 1:
            emit_group_tail(g)


def _split_multi_waits(nc):
    """This walrus build allows only one sem wait on several instruction
    structs (e.g. self-loading Matmult). Move extra waits onto preceding
    same-engine NOPs; engine FIFO order makes this equivalent."""
    n = 0
    for fn in nc.m.functions:
        for bb in fn.blocks:
            out = []
            for inst in bb.instructions:
                si = inst.sync_info
                if si is not None and si.on_wait and len(si.on_wait) > 1:
                    waits = list(si.on_wait)
                    for w in waits[:-1]:
                        n += 1
                        out.append(
                            mybir.InstNoOp(
                                name=f"nopw-{n}-{inst.name}",
                                engine=inst.engine,
                                sync_info=mybir.SyncInfo(
                                    on_wait=[w], on_update=[]
                                ),
                                bass_nofuse=True,
                            )
                        )
                    inst.sync_info = mybir.SyncInfo(
                        on_wait=[waits[-1]], on_update=list(si.on_update)
                    )
                out.append(inst)
            bb.instructions = out


_CACHE = {}


def _get_nc(with_bias: bool, loop_n: int = 1):
    key = (with_bias, loop_n, MM_BF16, PIECES, LOAD_ENG, STORE_ENG, MODE,
           STORE_GRP, RING_SPLIT, SCALE_SPLIT, GRP, ISSUE_LAZY, OUT_BF16)
    if key not in _CACHE:
        _patch_tile_drain()
        nc = _build(with_bias, loop_n)
        _split_multi_waits(nc)
        _CACHE[key] = nc
    return _CACHE[key]


def kernel(input, W, b, **run_kwargs):
    input = np.ascontiguousarray(np.asarray(input, dtype=np.float32))
    W = np.asarray(W, dtype=np.float32)
    b = np.asarray(b, dtype=np.float32)
    assert input.shape == (B, D) and W.shape == (L, D) and b.shape == (L, D)

    with_bias = bool(np.any(b))
    nc = _get_nc(with_bias)

    wt = np.ascontiguousarray(W.T)  # [D, L]
    in_maps = []
    for i in range(NCORES):
        m = {"x": input[i * R : (i + 1) * R], "wt": wt}
        if with_bias:
            # B_i = sum_{j<i} b_j ; beta_i = B_i . w_i ; B_L = sum_j b_j
            Bpre = np.concatenate(
                [np.zeros((1, D), np.float32), np.cumsum(b, axis=0)[:-1]], axis=0
            )
            m["bl"] = b.sum(axis=0, dtype=np.float32).reshape(1, D)
            m["beta"] = np.einsum("ld,ld->l", Bpre, W).astype(np.float32).reshape(1, L)
        in_maps.append(m)

    res = run_bass_kernel_spmd(
        nc, in_maps, core_ids=list(range(NCORES)), **run_kwargs
    )
    out = np.concatenate(
        [
            np.asarray(res.results[i]["y"]).astype(np.float32, copy=False)
            for i in range(NCORES)
        ],
        axis=0,
    )
    if run_kwargs:
        return out, res
    return out


# revision 52
# speedup vs baseline: 1.5254x; 1.0345x over previous
"""Trainium2 Bass kernel for nn_CrossNetwork (DCN-v1 cross network).

Math: reference computes x_{i+1} = input * (x_i . w_i) + x_i + b_i, L=6 layers.
Writing x_i = input * c_i + B_i with B_i = sum_{j<i} b_j (a constant row
vector) and c_i a per-row scalar, the recursion collapses to
    u_i    = input . w_i                     (per row, one tall-skinny matmul)
    beta_i = B_i . w_i                       (host-computed constants)
    c_{i+1} = c_i * (1 + u_i) + beta_i ; c_0 = 1
    out    = input * c_L + B_L
For the b == 0 case this is out = input * prod_i(1 + u_i).

Device work per core (2048 rows): load x once, PE-transpose 128x128 blocks
(f32r, 1.5 cyc/row), f32r matmul against W^T accumulating U^T[6, rows]
(1 cyc/row), DVE product-reduce to c, DVE per-partition-scalar multiply,
store. HBM traffic 16MB/core => memory-bound, roofline ~47us at 358GB/s.

Perf structure vs the original baseline (70.9us -> ~58-60us measured):
  - bf16 matmul inputs (fp32 matmul is 4 cyc/row on PE, bf16 is 1); the
    cast rides the ACT PSUM->SBUF copy, W cast once at startup. Dot
    products still accumulate in fp32 PSUM; global rel err ~5e-3 vs the
    2e-2 gate.
  - transpose(k+1) emitted before matmul(k) so the PE never stalls on the
    ACT PSUM->SBUF copy of transpose k.
  - DMA batched: 4x2MB loads (PIECES=4), 4x2MB stores (STORE_GRP=1)
    instead of 32 per-chunk 512KB transfers; measured best on HW
    (44-56us vs 58-60 for 2x4MB loads).
  - c = prod(1+u_i) via a single DVE mult-reduce off PSUM.
"""

import os

import numpy as np

import concourse.bass as bass
import concourse.mybir as mybir
import concourse.tile as tile
from concourse.bass_utils import run_bass_kernel_spmd
from concourse.masks import make_identity
from concourse.vector_clock import ScopedClock

F32 = mybir.dt.float32
BF16 = mybir.dt.bfloat16

B, D, L = 16384, 1024, 6
NCORES = 8
R = B // NCORES  # rows per core
P = 128
NCH = R // P  # chunks of 128 rows per core
KB = D // P  # 128-wide k blocks
# chunks per psum accumulation group (matmul N = GRP*128; 4 => 512, the
# fp32 PSUM bank cap)
GRP = int(os.environ.get("K_GRP", "4"))
NG = NCH // GRP

# --- tuning knobs (env-overridable for A/B; defaults are the shipped config)
MM_BF16 = os.environ.get("K_MM_BF16", "1") == "1"
PIECES = int(os.environ.get("K_PIECES", "4"))  # DMA pieces per direction
LOAD_ENG = os.environ.get("K_LOAD_ENG", "sync")
STORE_ENG = os.environ.get("K_STORE_ENG", "sync")
# full | dma | load | load2 | store | dma_nodep
MODE = os.environ.get("K_MODE", "full")
# store granularity in units of GRP-chunk groups (1 => 2MB stores)
STORE_GRP = int(os.environ.get("K_STORE_GRP", "1"))
# alternate HWDGE rings per transfer: none | stores | loads | both
RING_SPLIT = os.environ.get("K_RING_SPLIT", "none")
# send odd-chunk row-scales to ACT instead of DVE
SCALE_SPLIT = os.environ.get("K_SCALE_SPLIT", "0") == "1"
# issue load piece h+2 after store h instead of all loads upfront, so the
# HWDGE ring interleaves reads and writes (requires GPC == GRP)
ISSUE_LAZY = os.environ.get("K_ISSUE_LAZY", "0") == "1"
# store y as bf16 (half the write traffic), upcast host-side; output
# quantization ~2e-3 rel err, inside the 2e-2 gate
OUT_BF16 = os.environ.get("K_OUT_BF16", "1") == "1"
# bf16 path only: store every 2 chunks (512KB) as soon as scaled
STORE_SUB = os.environ.get("K_STORE_SUB", "1") == "1"
GPC = NCH // PIECES  # chunks per DMA piece
assert NCH % PIECES == 0 and GPC % GRP == 0
assert (GPC // GRP) % STORE_GRP == 0  # stores must not span piece tiles


def _patch_tile_drain():
    """This walrus build rejects >1 sem wait on a CTRL (Drain) instruction.

    Tile's kernel-tail drain waits on every sem domain at once; split it into
    chained single-wait drains.
    """
    if getattr(tile.TileContext, "_drain_patched", False):
        return

    def _drain_and_barrier(self, tick_clock, wait_clock):
        gc = tick_clock.global_clock
        entries = [(proc, t) for proc, t in enumerate(gc) if t > 0]
        if not entries:
            self.nc.sync.drain()
        for proc, t in entries:
            sub = ScopedClock()
            sub.require_at_least(None, proc, t)
            drain_inst = self.nc.sync.drain()
            wait_clock.add_sem_waits(drain_inst.ins, sub)

        self.nc.all_engine_barrier()
        assert self.sems is not None
        popped = self.nc._tile_sem_poison_stack.pop()
        assert popped is self._sem_poison
        self.nc.clear_and_free_semaphores(list(self.sems.allocated().values()))

    tile.TileContext._drain_and_barrier = _drain_and_barrier
    tile.TileContext._drain_patched = True


def _build(with_bias: bool, loop_n: int = 1):
    nc = bass.Bass("TRN2")
    x_d = nc.dram_tensor("x", [R, D], F32, kind="ExternalInput")
    wt_d = nc.dram_tensor("wt", [D, L], F32, kind="ExternalInput")
    if with_bias:
        bl_d = nc.dram_tensor("bl", [1, D], F32, kind="ExternalInput")
        beta_d = nc.dram_tensor("beta", [1, L], F32, kind="ExternalInput")
    out_bf = OUT_BF16 and not with_bias and MODE == "full"
    y_d = nc.dram_tensor(
        "y", [R, D], BF16 if out_bf else F32, kind="ExternalOutput"
    )

    xv = x_d.rearrange("(p n) d -> p n d", p=P)  # [128, NCH, D]
    yv = y_d.rearrange("(p n) d -> p n d", p=P)
    wtv = wt_d.rearrange("(k p) s -> p k s", p=P)  # [128, KB, L]

    with tile.TileContext(nc) as tc:
        nwin = NCH // (STORE_GRP * GRP)
        with (
            tc.tile_pool(name="consts", bufs=1) as consts,
            tc.tile_pool(name="xch", bufs=PIECES) as xpool,
            tc.tile_pool(name="xt", bufs=4) as xtpool,
            tc.tile_pool(name="small", bufs=2 * GRP) as small,
            tc.tile_pool(name="ybf", bufs=min(nwin, 4)) as ypool,
            tc.tile_pool(name="pxt", bufs=4, space="PSUM") as pxt,
            tc.tile_pool(name="pu", bufs=2, space="PSUM") as pu,
        ):
            ident = consts.tile([P, P], F32)
            make_identity(nc, ident)
            src_sb = None
            if MODE in ("store", "dma_nodep"):
                src_sb = []
                for h in range(PIECES):
                    t = consts.tile([P, GPC, D], F32, name=f"src{h}")
                    nc.vector.memset(t, 1.0)
                    src_sb.append(t)
            ident6 = consts.tile([L, L], F32)
            make_identity(nc, ident6)
            wt_sb = consts.tile([P, KB, L], F32)
            nc.sync.dma_start(out=wt_sb, in_=wtv)
            if MM_BF16:
                wt_bf = consts.tile([P, KB, L], BF16)
                nc.scalar.copy(wt_bf, wt_sb)
                wt_sb = wt_bf
            bl_sb = beta_sb = None
            if with_bias:
                bl_sb = consts.tile([P, D], F32)
                nc.sync.dma_start(
                    out=bl_sb,
                    in_=bass.AP(tensor=bl_d, offset=0, ap=[[0, P], [1, D]]),
                )
                beta_sb = consts.tile([P, L], F32)
                nc.sync.dma_start(
                    out=beta_sb,
                    in_=bass.AP(tensor=beta_d, offset=0, ap=[[0, P], [1, L]]),
                )

            import contextlib
            loop_cm = (
                tc.For_i(0, loop_n, 1) if loop_n > 1 else contextlib.nullcontext()
            )
            with loop_cm:
                _body(nc, tc, xpool, xtpool, small, ypool, pxt, pu, ident,
                      ident6, wt_sb, bl_sb, beta_sb, xv, yv, with_bias,
                      src_sb, out_bf)
    return nc


def _body(nc, tc, xpool, xtpool, small, ypool, pxt, pu, ident, ident6,
          wt_sb, bl_sb, beta_sb, xv, yv, with_bias, src_sb=None,
          out_bf=False):
    load_eng = getattr(nc, LOAD_ENG)
    store_eng = getattr(nc, STORE_ENG)

    if MODE in ("store", "dma_nodep"):
        # stores sourced from preset const tiles — no dependency coupling
        if MODE == "dma_nodep":
            for h in range(PIECES):
                t = xpool.tile([P, GPC, D], F32, tag="xch")
                load_eng.dma_start(
                    out=t, in_=xv[:, h * GPC : (h + 1) * GPC, :]
                )
        for h in range(PIECES):
            store_eng.dma_start(
                out=yv[:, h * GPC : (h + 1) * GPC, :], in_=src_sb[h]
            )
            if MODE == "store":
                store_eng.dma_start(
                    out=yv[:, h * GPC : (h + 1) * GPC, :], in_=src_sb[h]
                )
        return

    # big-piece loads: PIECES tiles of [128, GPC, D]
    lazy = ISSUE_LAZY and GPC == GRP and MODE == "full"
    x_sb = [None] * PIECES

    def load_piece(h):
        t = xpool.tile([P, GPC, D], F32, tag="xch")
        eng = load_eng
        if MODE == "load2" or RING_SPLIT in ("loads", "both"):
            eng = (nc.sync, nc.scalar)[h % 2]
        eng.dma_start(out=t, in_=xv[:, h * GPC : (h + 1) * GPC, :])
        x_sb[h] = t

    for h in range(2 if lazy else PIECES):
        load_piece(h)

    if MODE in ("load", "load2"):
        # 16MB of pure reads (two passes over the 8MB input)
        for h in range(PIECES):
            eng = load_eng if MODE != "load2" else (nc.sync, nc.scalar)[h % 2]
            eng.dma_start(out=x_sb[h], in_=xv[:, h * GPC : (h + 1) * GPC, :])
        return

    if MODE == "dma":
        for h in range(PIECES):
            store_eng.dma_start(
                out=yv[:, h * GPC : (h + 1) * GPC, :], in_=x_sb[h]
            )
        return

    def xch(n):
        # chunk n as a [128, D] view into its piece tile
        return x_sb[n // GPC][:, n % GPC, :]

    # software pipeline: emit transposes for (g,k) one step ahead of the
    # matmul consuming (g,k-1)'s copied tile, so PE never waits on ACT.
    flat = [(g, k) for g in range(NG) for k in range(KB)]
    xt_tiles = {}
    xt_dt = BF16 if MM_BF16 else F32

    def emit_transpose_copy(g, k):
        pxt_t = pxt.tile([P, GRP * P], F32, tag="pxt")
        for j in range(GRP):
            src = xch(g * GRP + j)[:, k * P : (k + 1) * P]
            nc.tensor.transpose(pxt_t[:, j * P : (j + 1) * P], src, ident)
        xt_t = xtpool.tile([P, GRP * P], xt_dt, tag="xt")
        nc.scalar.copy(xt_t, pxt_t)
        xt_tiles[(g, k)] = xt_t

    ut_tiles = {}

    def emit_matmul(g, k):
        if k == 0:
            ut_tiles[g] = pu.tile([L, GRP * P], F32, tag="u", name=f"ut{g}")
        lhs = wt_sb[:, k, :]
        rhs = xt_tiles.pop((g, k))[:]
        nc.tensor.matmul(
            ut_tiles[g][:], lhs, rhs, start=(k == 0), stop=(k == KB - 1)
        )

    ywin = {}

    def emit_group_tail(g):
        # 1 + U^T on DVE while copying PSUM->SBUF, transpose [6,128] blocks
        # back to row-major [128,6], product-reduce to c, scale rows.
        w, slot = g // STORE_GRP, g % STORE_GRP
        if out_bf and slot == 0:
            ywin[w] = ypool.tile(
                [P, STORE_GRP * GRP, D], BF16, tag="ybf", name=f"yw{w}"
            )
        ut_ps = ut_tiles.pop(g)
        u1t_t = xtpool.tile([L, GRP * P], F32, tag="u1t")
        nc.vector.tensor_scalar_add(u1t_t, ut_ps, 1.0)
        uj_ps = pu.tile([P, GRP, L], F32, tag="uj", name=f"uj{g}")
        for j in range(GRP):
            nc.tensor.transpose(
                uj_ps[:, j, :], u1t_t[:, j * P : (j + 1) * P], ident6
            )
        for j in range(GRP):
            n = g * GRP + j
            if not with_bias:
                # c = prod over the 6 (1+u_i): one mult-reduce off PSUM
                c_t = small.tile([P, 1], F32, tag="c")
                nc.vector.tensor_reduce(
                    c_t,
                    uj_ps[:, j, :],
                    axis=mybir.AxisListType.X,
                    op=mybir.AluOpType.mult,
                )
                y_ap = ywin[w][:, slot * GRP + j, :] if out_bf else xch(n)
                if SCALE_SPLIT and j % 2 == 1:
                    nc.scalar.activation(
                        y_ap,
                        xch(n),
                        mybir.ActivationFunctionType.Copy,
                        scale=c_t[:],
                    )
                else:
                    nc.vector.tensor_scalar_mul(y_ap, xch(n), c_t)
                if out_bf and STORE_SUB and j % 2 == 1:
                    c0 = slot * GRP + j - 1
                    store_eng.dma_start(
                        out=yv[:, n - 1 : n + 1, :],
                        in_=ywin[w][:, c0 : c0 + 2, :],
                    )
            else:
                u1_t = small.tile([P, L], F32, tag="u1")
                nc.vector.tensor_copy(u1_t, uj_ps[:, j, :])
                c_t = small.tile([P, 1], F32, tag="c")
                nc.vector.memset(c_t, 1.0)
                for i in range(L):
                    # c = c * (1 + u_i) + beta_i
                    nc.vector.scalar_tensor_tensor(
                        out=c_t,
                        in0=c_t,
                        scalar=u1_t[:, i : i + 1],
                        in1=beta_sb[:, i : i + 1],
                        op0=mybir.AluOpType.mult,
                        op1=mybir.AluOpType.add,
                    )
                # out = x * c + B_L
                nc.vector.scalar_tensor_tensor(
                    out=xch(n),
                    in0=xch(n),
                    scalar=c_t,
                    in1=bl_sb,
                    op0=mybir.AluOpType.mult,
                    op1=mybir.AluOpType.add,
                )
        if out_bf and STORE_SUB:
            if (g + 1) % STORE_GRP == 0:
                ywin.pop(g // STORE_GRP)
            return
        # store once STORE_GRP groups are scaled
        if (g + 1) % STORE_GRP == 0:
            s = g // STORE_GRP  # store index
            n0 = (g + 1 - STORE_GRP) * GRP  # first chunk of this store
            n1 = (g + 1) * GRP
            h = n0 // GPC
            eng = store_eng
            if RING_SPLIT in ("stores", "both"):
                eng = (nc.sync, nc.scalar)[s % 2]
            src = (
                ywin.pop(s)
                if out_bf
                else x_sb[h][:, n0 - h * GPC : n1 - h * GPC, :]
            )
            eng.dma_start(out=yv[:, n0:n1, :], in_=src)
            if lazy and s + 2 < PIECES:
                load_piece(s + 2)

    # pipelined emission
    emit_transpose_copy(*flat[0])
    for i, (g, k) in enumerate(flat):
        if i + 1 < len(flat):
            emit_transpose_copy(*flat[i + 1])
        emit_matmul(g, k)
        if k == KB - 1:
            emit_group_tail(g)


def _split_multi_waits(nc):
    """This walrus build allows only one sem wait on several instruction
    structs (e.g. self-loading Matmult). Move extra waits onto preceding
    same-engine NOPs; engine FIFO order makes this equivalent."""
    n = 0
    for fn in nc.m.functions:
        for bb in fn.blocks:
            out = []
            for inst in bb.instructions:
                si = inst.sync_info
                if si is not None and si.on_wait and len(si.on_wait) > 1:
                    waits = list(si.on_wait)
                    for w in waits[:-1]:
                        n += 1
                        out.append(
                            mybir.InstNoOp(
                                name=f"nopw-{n}-{inst.name}",
                                engine=inst.engine,
                                sync_info=mybir.SyncInfo(
                                    on_wait=[w], on_update=[]
                                ),
                                bass_nofuse=True,
                            )
                        )
                    inst.sync_info = mybir.SyncInfo(
                        on_wait=[waits[-1]], on_update=list(si.on_update)
                    )
                out.append(inst)
            bb.instructions = out


_CACHE = {}


def _get_nc(with_bias: bool, loop_n: int = 1):
    key = (with_bias, loop_n, MM_BF16, PIECES, LOAD_ENG, STORE_ENG, MODE,
           STORE_GRP, RING_SPLIT, SCALE_SPLIT, GRP, ISSUE_LAZY, OUT_BF16,
           STORE_SUB)
    if key not in _CACHE:
        _patch_tile_drain()
        nc = _build(with_bias, loop_n)
        _split_multi_waits(nc)
        _CACHE[key] = nc
    return _CACHE[key]


def kernel(input, W, b, **run_kwargs):
    input = np.ascontiguousarray(np.asarray(input, dtype=np.float32))
    W = np.asarray(W, dtype=np.float32)
    b = np.asarray(b, dtype=np.float32)
    assert input.shape == (B, D) and W.shape == (L, D) and b.shape == (L, D)

    with_bias = bool(np.any(b))
    nc = _get_nc(with_bias)

    wt = np.ascontiguousarray(W.T)  # [D, L]
    in_maps = []
    for i in range(NCORES):
        m = {"x": input[i * R : (i + 1) * R], "wt": wt}
        if with_bias:
            # B_i = sum_{j<i} b_j ; beta_i = B_i . w_i ; B_L = sum_j b_j
            Bpre = np.concatenate(
                [np.zeros((1, D), np.float32), np.cumsum(b, axis=0)[:-1]], axis=0
            )
            m["bl"] = b.sum(axis=0, dtype=np.float32).reshape(1, D)
            m["beta"] = np.einsum("ld,ld->l", Bpre, W).astype(np.float32).reshape(1, L)
        in_maps.append(m)

    res = run_bass_kernel_spmd(
        nc, in_maps, core_ids=list(range(NCORES)), **run_kwargs
    )
    out = np.concatenate(
        [
            np.asarray(res.results[i]["y"]).astype(np.float32, copy=False)
            for i in range(NCORES)
        ],
        axis=0,
    )
    if run_kwargs:
        return out, res
    return out
